# revision 13
# baseline (speedup 1.0000x reference)
"""Trainium2 Bass kernel for MlpLSTM: MLP -> LSTM scan -> value/logits heads.

Full inputs in, full outputs out. Data-parallel over batch: 256 / 8 cores = 32
sequences per core. All matmuls in bf16 with fp32 PSUM accumulation.

Per-core pipeline (batch shard b=32, S=128, F=512, H=1024, A=18, T=b*S tokens):
  GEMM1: xT[h, tok] = relu(W1 @ obs.T + b1)          (token-major, M=128)
  GEMM2: xg[tok, 4H] = x @ W_ih.T + (b_ih + b_hh)    -> DRAM scratch
  Scan:  gates[b, 4H] = xg_t + h @ W_hh.T, batch-stationary matmul with 4x
         column tiling (4 col-groups of the PE array, one per 512-wide gate
         chunk, packed into one PSUM bank across partition groups).
         Gate nonlinearities in a [128, 512] packed layout (i|f and g|o
         stacked along partitions) so DVE/ACT use all 128 lanes.
         h is transposed back via PE-transpose into an SBUF-resident history
         hsT[k][128, S*32] that feeds both the next step and the epilogue.
  Epilogue: [logits|value] = hs @ [W_l|W_v].T, fused log-softmax, entropy,
         and one-hot gather of behaviour log-probs; output [T, 4] f32.
"""

import numpy as np
import ml_dtypes

B, S_FULL, F, H, A = 256, 128, 512, 1024, 18
NC = 8
BS = B // NC  # 32 sequences per core
G4 = 4 * H    # 4096

_compiled = {}


def _build(S, phases=(1, 2, 3, 4, 7)):
    import concourse.bacc as bacc
    import concourse.bass as bass
    import concourse.tile as tile
    import concourse.mybir as mybir

    f32 = mybir.dt.float32
    bf16 = mybir.dt.bfloat16
    ts = bass.ts
    AF = mybir.ActivationFunctionType
    ALU = mybir.AluOpType
    AX = mybir.AxisListType

    T = BS * S               # tokens per core
    NT = T // 512            # 512-token chunks (GEMM1 N)
    NM = T // 128            # 128-token chunks (GEMM2 M, epilogue)

    nc = bacc.Bacc("TRN2", target_bir_lowering=False, debug=False, num_devices=NC)

    def din(name, shape, dt):
        return nc.dram_tensor(name, shape, dt, kind="ExternalInput").ap()

    obsT = din("obsT", [4, 128, T], bf16)          # F=4 k-tiles of obs.T
    w1T = din("w1T", [4, 128, H], bf16)            # W1.T k-tiles
    b1 = din("b1", [8, 128, 1], f32)
    wihT = din("wihT", [8, 128, G4], bf16)         # W_ih.T k-tiles
    biasg = din("biasg", [128, G4], f32)           # (b_ih+b_hh) bcast over partitions
    whhT = din("whhT", [8, 128, G4], bf16)         # W_hh.T k-tiles
    hxT = din("hxT", [128, 8 * BS], bf16)          # initial h, transposed+packed
    cx0 = din("cx0", [64, 512], f32)               # initial c, packed halves
    ident = din("ident", [64, 32], f32)            # I32 stacked twice
    wvlT = din("wvlT", [8, 128, A + 1], bf16)      # [W_l|W_v].T k-tiles
    bvl = din("bvl", [128, A + 1], f32)
    onehot = din("onehot", [T, A], f32)            # (t,b) token order
    xg = nc.dram_tensor("xg", [S, 2, 4, BS, 512], f32).ap()   # scratch
    out = nc.dram_tensor("out", [T, 4], f32, kind="ExternalOutput").ap()

    with tile.TileContext(nc) as tc:
        with tc.tile_pool(name="consts", bufs=1) as consts:
            ident_sb = consts.tile([64, 32], f32)
            nc.sync.dma_start(ident_sb[:], ident[:])
            b1_sb = []
            for m in range(8):
                t_ = consts.tile([128, 1], f32, tag=f"b1_{m}")
                nc.sync.dma_start(t_[:], b1[m])
                b1_sb.append(t_)
            bvl_sb = consts.tile([128, A + 1], f32)
            nc.sync.dma_start(bvl_sb[:], bvl[:])

            # ---------------- Phase 1+2: MLP and input projection ----------
            if 4 not in phases:
                zo = consts.tile([128, 4], f32)
                nc.vector.memset(zo[:], 0.0)
                for m in range(NM):
                    nc.sync.dma_start(out[ts(m, 128), :], zo[:])
            with tc.tile_pool(name="xt", bufs=1) as xt_pool:
                xT = [xt_pool.tile([128, T], bf16, tag=f"xt{m}", name=f"xT{m}") for m in range(8)]

                with (
                    tc.tile_pool(name="g1", bufs=1) as g1,
                    tc.tile_pool(name="ps1", bufs=4, space="PSUM") as ps1,
                ):
                    obsT_sb = []
                    w1T_sb = []
                    for k in range(4):
                        o_ = g1.tile([128, T], bf16, tag=f"obsT{k}")
                        nc.sync.dma_start(o_[:], obsT[k])
                        obsT_sb.append(o_)
                        w_ = g1.tile([128, H], bf16, tag=f"w1T{k}")
                        nc.sync.dma_start(w_[:], w1T[k])
                        w1T_sb.append(w_)
                    for m in range(8 if 1 in phases else 0):
                        for n in range(NT):
                            ps = ps1.tile([128, 512], f32, tag="ps1")
                            for k in range(4):
                                nc.tensor.matmul(
                                    ps[:],
                                    w1T_sb[k][:, ts(m, 128)],
                                    obsT_sb[k][:, ts(n, 512)],
                                    start=(k == 0),
                                    stop=(k == 3),
                                )
                            nc.scalar.activation(
                                xT[m][:, ts(n, 512)], ps[:], AF.Relu, bias=b1_sb[m]
                            )

                with (
                    tc.tile_pool(name="g2", bufs=1) as g2,
                    tc.tile_pool(name="g2w", bufs=4) as g2w,
                    tc.tile_pool(name="ps2", bufs=4, space="PSUM") as ps2,
                ):
                    wihT_sb = []
                    for k in range(8):
                        w_ = g2.tile([128, G4], bf16, tag=f"wihT{k}")
                        nc.sync.dma_start(w_[:], wihT[k])
                        wihT_sb.append(w_)
                    biasg_sb = g2.tile([128, G4], f32)
                    nc.sync.dma_start(biasg_sb[:], biasg[:])
                    for m in range(NM if 2 in phases else 0):
                        t0 = m * (128 // BS)
                        for n in range(8):
                            ps = ps2.tile([128, 512], f32, tag="ps2")
                            for k in range(8):
                                nc.tensor.matmul(
                                    ps[:],
                                    xT[k][:, ts(m, 128)],
                                    wihT_sb[k][:, ts(n, 512)],
                                    start=(k == 0),
                                    stop=(k == 7),
                                )
                            xg_sb = g2w.tile([128, 512], f32, tag="xgw")
                            nc.vector.tensor_add(
                                xg_sb[:], ps[:], biasg_sb[:, ts(n, 512)]
                            )
                            # dst is (t, b, f) with gaps between t rows; DMA only
                            # needs matching element counts and iteration order
                            dst = xg[t0 : t0 + 128 // BS, n // 4, n % 4, :, :]
                            nc.sync.dma_start(dst, xg_sb[:])

            # ---------------- Phase 3: LSTM scan ---------------------------
            with tc.tile_pool(name="hst", bufs=1) as hst_pool:
                hsT = [
                    hst_pool.tile([128, T], bf16, tag=f"hsT{k}", name=f"hsT{k}")
                    for k in range(8)
                ]

                with (
                    tc.tile_pool(name="whh", bufs=1) as whh_pool,
                    tc.tile_pool(name="sc", bufs=1) as sc,
                    tc.tile_pool(name="scw", bufs=2) as scw,
                    tc.tile_pool(name="psg", bufs=2, space="PSUM") as psg,
                    tc.tile_pool(name="psh", bufs=2, space="PSUM") as psh,
                ):
                    whhT_sb = []
                    for k in range(8):
                        w_ = whh_pool.tile([128, G4], bf16, tag=f"whhT{k}")
                        nc.sync.dma_start(w_[:], whhT[k])
                        whhT_sb.append(w_)
                    hT0 = sc.tile([128, 8 * BS], bf16)
                    nc.sync.dma_start(hT0[:], hxT[:])
                    c_sb = sc.tile([64, 512], f32)
                    nc.sync.dma_start(c_sb[:], cx0[:])

                    for t in range(S if 3 in phases else 0):
                        # gate matmuls: 2 PSUM banks x 4 col-groups x 8 k-tiles
                        pa = psg.tile([128, 512], f32, tag="pga")
                        pb = psg.tile([128, 512], f32, tag="pgb")
                        banks = (pa, pb)
                        for k in range(8):
                            if t == 0:
                                lhsT = hT0[:, ts(k, BS)]
                            else:
                                lhsT = hsT[k][:, ts(t - 1, BS)]
                            for half in range(2):
                                for j in range(4):
                                    n = half * 4 + j
                                    nc.tensor.matmul(
                                        banks[half][ts(j, 32), :],
                                        lhsT,
                                        whhT_sb[k][:, ts(n, 512)],
                                        start=(k == 0),
                                        stop=(k == 7),
                                        tile_position=(0, 32 * j),
                                        skip_group_check=True,
                                    )
                        # unpack psum + xg into one base-0 [64,512] tile per gate
                        # (two SBUF inputs must share base partition; the PSUM
                        # input may sit at base 64)
                        xga = scw.tile([128, 512], f32, tag="xga")
                        nc.sync.dma_start(
                            xga[:], xg[t, 0].rearrange("j b f -> (j b) f")
                        )
                        xgb = scw.tile([128, 512], f32, tag="xgb")
                        nc.sync.dma_start(
                            xgb[:], xg[t, 1].rearrange("j b f -> (j b) f")
                        )
                        gi = scw.tile([64, 512], f32, tag="gi")
                        nc.vector.tensor_add(gi[:], pa[0:64, :], xga[0:64, :])
                        gf = scw.tile([64, 512], f32, tag="gf")
                        nc.vector.tensor_add(gf[:], pa[64:128, :], xga[64:128, :])
                        gg = scw.tile([64, 512], f32, tag="gg")
                        nc.vector.tensor_add(gg[:], pb[0:64, :], xgb[0:64, :])
                        go = scw.tile([64, 512], f32, tag="go")
                        nc.vector.tensor_add(go[:], pb[64:128, :], xgb[64:128, :])
                        si = scw.tile([64, 512], f32, tag="si")
                        nc.scalar.activation(si[:], gi[:], AF.Sigmoid)
                        sf = scw.tile([64, 512], f32, tag="sf")
                        nc.scalar.activation(sf[:], gf[:], AF.Sigmoid)
                        sg_ = scw.tile([64, 512], f32, tag="sg")
                        nc.scalar.activation(sg_[:], gg[:], AF.Tanh)
                        so_ = scw.tile([64, 512], f32, tag="so")
                        nc.scalar.activation(so_[:], go[:], AF.Sigmoid)
                        # c = f*c + i*g~ ; h = o * tanh(c)
                        t1 = scw.tile([64, 512], f32, tag="t1")
                        nc.vector.tensor_mul(t1[:], si[:], sg_[:])
                        t2 = scw.tile([64, 512], f32, tag="t2")
                        nc.vector.tensor_mul(t2[:], sf[:], c_sb[:])
                        nc.vector.tensor_add(c_sb[:], t1[:], t2[:])
                        tch = scw.tile([64, 512], f32, tag="tch")
                        nc.scalar.activation(tch[:], c_sb[:], AF.Tanh)
                        # h in two base-0 tiles: PE-transpose inputs must sit at
                        # base partition 0 (base-32 transposes wedge the device)
                        h_lo = scw.tile([32, 512], f32, tag="h_lo")
                        nc.vector.tensor_mul(h_lo[:], so_[0:32, :], tch[0:32, :])
                        h_hi = scw.tile([32, 512], f32, tag="h_hi")
                        nc.vector.tensor_mul(h_hi[:], so_[32:64, :], tch[32:64, :])
                        # transpose h back to [H, b] layout, store into history
                        if 5 in phases:
                            for k in range(8):
                                nc.vector.memset(hsT[k][:, ts(t, BS)], 0.0)
                        else:
                            hhalves = (h_lo, h_hi)
                            ph = psh.tile([128, 8 * 32], f32, tag="ph")
                            for k in range(8):
                                nc.tensor.transpose(
                                    ph[:, ts(k, 32)],
                                    hhalves[k // 4][:, ts(k % 4, 128)],
                                    ident_sb[0:32, :],
                                )
                            for k in range(8):
                                nc.vector.tensor_copy(
                                    hsT[k][:, ts(t, BS)], ph[:, ts(k, 32)]
                                )

                # ---------------- Phase 4: heads + log-softmax -------------
                with (
                    tc.tile_pool(name="epi", bufs=1) as epi,
                    tc.tile_pool(name="epw", bufs=3) as epw,
                    tc.tile_pool(name="pse", bufs=2, space="PSUM") as pse,
                ):
                    wvlT_sb = []
                    for k in range(8):
                        w_ = epi.tile([128, A + 1], bf16, tag=f"wvlT{k}")
                        nc.sync.dma_start(w_[:], wvlT[k])
                        wvlT_sb.append(w_)
                    for m in range(NM if 4 in phases else 0):
                        ps = pse.tile([128, A + 1], f32, tag="pse")
                        for k in range(8):
                            nc.tensor.matmul(
                                ps[:],
                                hsT[k][:, ts(m, 128)],
                                wvlT_sb[k][:],
                                start=(k == 0),
                                stop=(k == 7),
                            )
                        lg = epw.tile([128, A + 1], f32, tag="lg")
                        nc.vector.tensor_add(lg[:], ps[:], bvl_sb[:])
                        oh = epw.tile([128, A], f32, tag="oh")
                        nc.sync.dma_start(oh[:], onehot[ts(m, 128), :])
                        mx = epw.tile([128, 1], f32, tag="mx")
                        nc.vector.reduce_max(mx[:], lg[:, 0:A], axis=AX.X)
                        nmx = epw.tile([128, 1], f32, tag="nmx")
                        nc.vector.tensor_scalar_mul(nmx[:], mx[:], -1.0)
                        d = epw.tile([128, A], f32, tag="d")
                        nc.scalar.activation(d[:], lg[:, 0:A], AF.Identity, bias=nmx[:])
                        e = epw.tile([128, A], f32, tag="e")
                        z = epw.tile([128, 1], f32, tag="z")
                        if 8 in phases:
                            nc.scalar.activation(e[:], lg[:, 0:A], AF.Exp, bias=nmx[:])
                            nc.vector.reduce_sum(z[:], e[:], axis=AX.X)
                        else:
                            nc.scalar.activation(
                                e[:], lg[:, 0:A], AF.Exp, bias=nmx[:], accum_out=z[:]
                            )
                        lnz = epw.tile([128, 1], f32, tag="lnz")
                        nc.scalar.activation(lnz[:], z[:], AF.Ln)
                        ed = epw.tile([128, A], f32, tag="ed")
                        s2 = epw.tile([128, 1], f32, tag="s2")
                        od = epw.tile([128, A], f32, tag="od")
                        dsel = epw.tile([128, 1], f32, tag="dsel")
                        if 7 in phases:
                            nc.vector.tensor_mul(ed[:], e[:], d[:])
                            nc.vector.reduce_sum(s2[:], ed[:], axis=AX.X)
                            nc.vector.tensor_mul(od[:], oh[:], d[:])
                            nc.vector.reduce_sum(dsel[:], od[:], axis=AX.X)
                        else:
                            nc.vector.tensor_tensor_reduce(
                                ed[:], e[:], d[:], 1.0, 0.0, ALU.mult, ALU.add, s2[:]
                            )
                            nc.vector.tensor_tensor_reduce(
                                od[:], oh[:], d[:], 1.0, 0.0, ALU.mult, ALU.add, dsel[:]
                            )
                        zr = epw.tile([128, 1], f32, tag="zr")
                        nc.vector.reciprocal(zr[:], z[:])
                        s2z = epw.tile([128, 1], f32, tag="s2z")
                        nc.vector.tensor_mul(s2z[:], s2[:], zr[:])
                        ob = epw.tile([128, 4], f32, tag="ob")
                        nc.vector.tensor_sub(ob[:, 0:1], dsel[:], lnz[:])
                        nc.vector.tensor_sub(ob[:, 1:2], lnz[:], s2z[:])
                        nc.vector.tensor_copy(ob[:, 2:3], lg[:, A : A + 1])
                        nc.vector.tensor_copy(ob[:, 3:4], lnz[:])
                        nc.sync.dma_start(out[ts(m, 128), :], ob[:])

    nc.compile()
    return nc


def _prep_inputs(S, obs, hx, cx, behaviour_acts, W1, b1, W_ih, b_ih, W_hh, b_hh,
                 W_v, b_v, W_l, b_l):
    bf = ml_dtypes.bfloat16
    f32 = np.float32
    T = BS * S

    def c_(a):
        return np.ascontiguousarray(a)

    # weights shared by every core
    w1T = c_(W1.T.reshape(4, 128, H).astype(bf))
    b1r = c_(b1.reshape(8, 128, 1).astype(f32))
    wihT = c_(W_ih.T.reshape(8, 128, G4).astype(bf))
    bias = (b_ih + b_hh).astype(f32)
    biasg = c_(np.broadcast_to(bias[None, :], (128, G4)))
    whhT = c_(W_hh.T.reshape(8, 128, G4).astype(bf))
    ident = c_(np.tile(np.eye(32, dtype=f32), (2, 1)))
    wvl = np.concatenate([W_l, W_v], axis=0)          # [19, H]
    wvlT = c_(wvl.T.reshape(8, 128, A + 1).astype(bf))
    bvl = c_(np.broadcast_to(
        np.concatenate([b_l, b_v]).astype(f32)[None, :], (128, A + 1)))

    shared = dict(w1T=w1T, b1=b1r, wihT=wihT, biasg=biasg, whhT=whhT,
                  ident=ident, wvlT=wvlT, bvl=bvl)

    in_maps = []
    acts = np.asarray(behaviour_acts).reshape(B, -1)[:, :S].astype(np.int64)
    for i in range(NC):
        bs = slice(i * BS, (i + 1) * BS)
        obs_c = obs[bs, :S]
        obsT = c_(obs_c.transpose(1, 0, 2).reshape(T, F).T.reshape(4, 128, T).astype(bf))
        hxT = c_(hx[bs].T.reshape(8, 128, BS).transpose(1, 0, 2).reshape(128, 8 * BS).astype(bf))
        cx_c = cx[bs].astype(f32)
        cx0 = c_(np.concatenate([cx_c[:, :512], cx_c[:, 512:]], axis=0))
        a_c = acts[bs]                                 # [BS, S]
        oh = np.zeros((S, BS, A), f32)
        oh[np.arange(S)[:, None], np.arange(BS)[None, :], a_c.T] = 1.0
        in_maps.append(dict(shared, obsT=obsT, hxT=hxT, cx0=cx0,
                            onehot=c_(oh.reshape(T, A))))
    return in_maps


def run_device(S, inputs, trace=False, trace_kwargs=None, phases=(1, 2, 3, 4, 7)):
    from concourse.bass_utils import run_bass_kernel_spmd

    key = (S, phases)
    if key not in _compiled:
        _compiled[key] = _build(S, phases)
    nc = _compiled[key]
    in_maps = _prep_inputs(S, **inputs)
    kw = {}
    if trace:
        kw = dict(trace=True, trace_kwargs=trace_kwargs or {})
    res = run_bass_kernel_spmd(nc, in_maps, list(range(NC)), **kw)
    T = BS * S
    logp = np.empty((B, S, 1), np.float32)
    ent = np.empty((B, S, 1), np.float32)
    val = np.empty((B, S, 1), np.float32)
    for i in range(NC):
        o = res.results[i]["out"].reshape(S, BS, 4).transpose(1, 0, 2)
        bs = slice(i * BS, (i + 1) * BS)
        logp[bs] = o[:, :, 0:1]
        ent[bs] = o[:, :, 1:2]
        val[bs] = o[:, :, 2:3]
    return (logp, ent, val), res


def kernel(obs, hx, cx, behaviour_acts, W1, b1, W_ih, b_ih, W_hh, b_hh,
           W_v, b_v, W_l, b_l):
    inputs = dict(
        obs=np.asarray(obs, np.float32), hx=np.asarray(hx, np.float32),
        cx=np.asarray(cx, np.float32),
        behaviour_acts=np.asarray(behaviour_acts),
        W1=np.asarray(W1, np.float32), b1=np.asarray(b1, np.float32),
        W_ih=np.asarray(W_ih, np.float32), b_ih=np.asarray(b_ih, np.float32),
        W_hh=np.asarray(W_hh, np.float32), b_hh=np.asarray(b_hh, np.float32),
        W_v=np.asarray(W_v, np.float32), b_v=np.asarray(b_v, np.float32),
        W_l=np.asarray(W_l, np.float32), b_l=np.asarray(b_l, np.float32),
    )
    (logp, ent, val), _ = run_device(S_FULL, inputs)
    return logp, ent, val
